# revision 1
# baseline (speedup 1.0000x reference)
"""Trainium2 Bass kernel for nn_DINA_25503515804209 (sparse_attention).

Math (per batch b, head h):
  M = concat(x1, pad(x2)) in R^{2048 x 64}
  K = (1/8) * M U_h M^T          (2048 x 2048)
  rows_i = max(0, max_{p in allowed(i)} K[i,p])
  cols_p = max(0, max_{i in allowed(p)} K[i,p])
    (leading 848x848 block masked; the reference's mask fill value
     min(relu(K_head0)) is 0 for any real input since relu >= 0 and some
     entry is always <= 0 -- the max(0, .) floor implements it exactly)
  alpha = rows + cols; w1 = softmax(alpha[:1200]); w2 = softmax(alpha[1200:])
  r1 = w1 @ M[:1200]; r2 = w2 @ M[1200:]

Sharding: data-parallel over batch B=8 across the 8 NeuronCores.
Per core: PE computes K strip tiles via two-stage f32r matmuls (both
heads packed at contraction-row offsets 0/64); the DVE drains each PSUM
strip with the custom TENSOR_MASK_REDUCE (masked row-max accumulation +
masked fp16 copy); fp16 tensor_max accumulates the column-max surface,
finalized by PE transposes + a reduce; softmax and the weighted sums
against M are a small tail (ACT exp + tiny matmuls).
"""

import json

import numpy as np

B, L1, D1, L2, D2, H, C = 8, 1200, 64, 848, 48, 2, 64
Q = L1 + L2            # 2048
NT = Q // 128          # 16 row tiles
MASKED = L2            # leading 848x848 block is masked

_CACHE = {}


# --------------------------------------------------------------------------
# BIR post-processing: this walrus build encodes at most one semaphore wait
# per instruction; Tile emits multi-wait sync_infos.  Hoist excess waits
# into preceding same-engine EventSemaphore instructions (what wait_ge
# emits) -- engine sequencers execute in order, so semantics are identical.
# Also run codegen_inst_isa_subclasses, which populates .instr bytes for
# InstISA subclasses (custom DVE ops); raw Bass does not run that pass and
# walrus fails with "ISA wrong length" on empty instr arrays.
# --------------------------------------------------------------------------
def _split_waits_json(j):
    for fn in j.get("functions", []):
        for blk in fn.get("blocks", []):
            insts = blk.get("instructions")
            if not insts:
                continue
            out = []
            for ins in insts:
                si = ins.get("sync_info")
                waits = (si or {}).get("on_wait") or []
                if len(waits) > 1:
                    for k, wt in enumerate(waits[:-1]):
                        out.append(
                            {
                                "debug": ins.get("debug"),
                                "engine": ins["engine"],
                                "ins": [],
                                "name": f"{ins['name']}_hw{k}",
                                "opcode": "EventSemaphore",
                                "outs": [],
                                "sync_info": {"on_update": [], "on_wait": [wt]},
                            }
                        )
                    si["on_wait"] = waits[-1:]
                ups = (si or {}).get("on_update") or []
                if len(ups) > 1:
                    raise RuntimeError(
                        f"instruction {ins['name']} has {len(ups)} updates"
                    )
                out.append(ins)
            blk["instructions"] = out


def _patch_bass_json(nc):
    import concourse.mybir as mybir

    orig = nc.to_json_bytes
    done = []

    def to_json_bytes_patched():
        if not done:
            mybir.codegen_inst_isa_subclasses(nc)
            done.append(True)
        j = json.loads(orig())
        _split_waits_json(j)
        return json.dumps(j).encode()

    nc.to_json_bytes = to_json_bytes_patched
    return nc


def _ttmax_reduce_op():
    """Fused  out = max(in0, in1);  accum_out = rowmax(out)  custom DVE op.

    Consumes two fp16 streams per cycle (both DVE read ports), so one
    instruction replaces the whole pairwise row-max tree of a strip.
    Registered at runtime through dve_ops' documented extension point
    (the uop program ships in the per-NEFF DVE table)."""
    import numpy as np
    import concourse.dve_ops as dve_ops
    from concourse.dve_spec import Spec, Src0, Src1, maxx, lower
    from concourse.dve_table_gen import dve_ver_for
    from concourse.dve_uop import DveOpSpec

    NAME = "TT_MAX_ROWMAX_ANT"
    if NAME in dve_ops._SUB_OPCODE_FOR_NAME:
        return next(op for op in dve_ops.OPS if op.name == NAME)

    def _ref(in0, in1, c0, c1, c2):
        body = np.maximum(in0.astype(np.float32), in1.astype(np.float32))
        return body, body.reshape(body.shape[0], -1).max(axis=-1, keepdims=True)

    spec = Spec(body=maxx(Src0, Src1), accum=maxx, reference=_ref)
    row = dve_ops._CUSTOM_DVE_ROW_BASE + len(dve_ops.OPS)
    ver = dve_ver_for("TRN2")
    sha = DveOpSpec(
        name=NAME, opcode=row, uops=lower(spec, ver=ver), rd1_en=True
    ).sha(ver)
    op = dve_ops.DveOp(NAME, spec, subdim=False, uops_sha={ver: sha})
    dve_ops.OPS.append(op)
    dve_ops._SUB_OPCODE_FOR_NAME[NAME] = row
    dve_ops.CUSTOM_DVE_SPECS[NAME] = spec
    return op


def _build_nc():
    import concourse.bass as bass
    import concourse.mybir as mybir
    import concourse.tile as tile
    from concourse.dve_ops import TENSOR_MASK_REDUCE
    from concourse.masks import make_identity

    ttmax = _ttmax_reduce_op()

    f32 = mybir.dt.float32
    f32r = mybir.dt.float32r
    f16 = mybir.dt.float16
    AX = mybir.AxisListType
    ALU = mybir.AluOpType
    ACTF = mybir.ActivationFunctionType

    nc = bass.Bass(trn_type="TRN2")

    mt_d = nc.dram_tensor("mt_in", [C, Q], f32, kind="ExternalInput")
    m_d = nc.dram_tensor("m_in", [Q, C], f32, kind="ExternalInput")
    at_d = nc.dram_tensor("at_in", [C, 2, Q], f32, kind="ExternalInput")
    sa_d = nc.dram_tensor("starta_in", [128, NT], f32, kind="ExternalInput")
    ea_d = nc.dram_tensor("enda_in", [128, NT], f32, kind="ExternalInput")
    bm_d = nc.dram_tensor("bmask_in", [128, 2], f32, kind="ExternalInput")
    out_d = nc.dram_tensor("out", [4, C], f32, kind="ExternalOutput")

    with tile.TileContext(nc) as tc:
        with (
            tc.tile_pool(name="sb", bufs=1) as sb,
            tc.tile_pool(name="escr", bufs=4) as escr,
        ):
            # ---- load inputs (f32r tiles loaded directly; PE rounds).
            # A^T = (M U_h)^T is precomputed on the host (33 MFLOP) so the
            # strip matmuls start as soon as the first DMA chunks land.
            # Order: what strip t0 (restricted, cols 848:) needs comes first.
            mtr = sb.tile([C, Q], f32r, tag="mtr")
            atr = sb.tile([C, 2, Q], f32r, tag="atr")
            nc.scalar.dma_start(
                out=atr[:, :, 0:512], in_=at_d[:, :, 0:512].bitcast(f32r)
            )
            for j in (1, 2, 3, 0):
                s = slice(512 * j, 512 * (j + 1))
                nc.sync.dma_start(out=mtr[:, s], in_=mt_d[:, s].bitcast(f32r))
            for j in (1, 2, 3):
                s = slice(512 * j, 512 * (j + 1))
                nc.scalar.dma_start(out=atr[:, :, s], in_=at_d[:, :, s].bitcast(f32r))

            e1200 = sb.tile([128, 1], f32, tag="e1200")
            nc.vector.memset(e1200, float(Q - MASKED))
            sa = sb.tile([128, NT], f32, tag="sa")
            ea = sb.tile([128, NT], f32, tag="ea")
            nc.sync.dma_start(out=sa, in_=sa_d[:, :])
            nc.sync.dma_start(out=ea, in_=ea_d[:, :])

            ident16 = sb.tile([128, 128], f16, tag="ident16")
            make_identity(nc, ident16)
            ident32 = sb.tile([128, 128], f32, tag="ident32")
            make_identity(nc, ident32)

            # ---- per-head: A^T prep, K strips, col-max finalize ----
            # Row tiles 0..5 lie fully inside the masked block: their first
            # 512 columns are always masked out, so skip bank 0 entirely.
            # The col-max surface is seeded with 0 (cols get a relu floor at
            # the end, so a 0 seed is exact).
            rows0 = sb.tile([128, NT], f32, tag="rows0")
            rows1 = sb.tile([128, NT], f32, tag="rows1")
            cols0 = sb.tile([128, NT], f32, tag="cols0")
            cols1 = sb.tile([128, NT], f32, tag="cols1")
            acc0 = sb.tile([128, Q], f16, tag="acc0")
            acc1 = sb.tile([128, Q], f16, tag="acc1")
            trA = sb.tile([128, Q // 2], f16, tag="trA")
            nc.vector.memset(acc0[:, 0:848], 0.0)
            nc.vector.memset(acc1[:, 0:848], 0.0)

            NRESTR = 6
            with tc.tile_pool(name="psK", bufs=1, space="PSUM") as psK:
                def strips(h):
                    acc = acc0 if h == 0 else acc1
                    rows = rows0 if h == 0 else rows1
                    for t in range(NT):
                        isl = slice(128 * t, 128 * (t + 1))
                        # restricted strips: every row is masked, so only the
                        # window [848:2048] matters -- drain it unmasked
                        lo = MASKED if t < NRESTR else 0
                        mmlo = 512 if t < NRESTR else 0
                        pkf = psK.tile([128, Q], f32, tag=f"pk{(t + 1) % 2}",
                                       name=f"pk_{h}_{t}")
                        pk = pkf[:, lo:Q]
                        for j in range(mmlo // 512, 4):
                            nc.tensor.matmul(
                                pkf[:, 512 * j : 512 * (j + 1)],
                                atr[:, h, isl],
                                mtr[:, 512 * j : 512 * (j + 1)],
                                start=True, stop=True,
                            )
                        if t == 0:
                            eout = acc[:, lo:Q]
                            efull = None
                        else:
                            efull = escr.tile([128, Q], f16, tag="e",
                                              name=f"e_{t}_{h}")
                            eout = efull[:, lo:Q]
                        if t == NRESTR or (h == 0 and t < 2):
                            # boundary tile (per-partition mask); also the
                            # first two strips, so the DVE has work while
                            # the input DMAs and first ACT copies ramp up
                            if t == NRESTR:
                                dr_s, dr_e = sa[:, t : t + 1], ea[:, t : t + 1]
                            else:
                                dr_s, dr_e = 0.0, e1200
                            nc.vector._custom_dve(
                                TENSOR_MASK_REDUCE,
                                out=eout,
                                in0=pk[:, :],
                                in1=dr_e,
                                s0=dr_s,
                                s1=0.0,
                                imm2=1.0,
                                accum_out=rows[:, t : t + 1],
                            )
                        else:
                            # unmasked strip: ACT drains PSUM -> fp16; DVE
                            # row-maxes the fp16 copy via a 2x TT-max tree
                            nc.scalar.copy(eout, pk[:, :])
                            w = (Q - lo) // 2
                            nc.vector._custom_dve(
                                ttmax,
                                out=trA[:, 0:w],
                                in0=eout[:, 0:w],
                                in1=eout[:, w : 2 * w],
                                accum_out=rows[:, t : t + 1],
                            )
                        if t > 0:
                            nc.vector.tensor_max(
                                acc[:, lo:Q], acc[:, lo:Q], efull[:, lo:Q]
                            )

                def finalize(h):
                    acc = acc0 if h == 0 else acc1
                    cols = cols0 if h == 0 else cols1
                    pt = psK.tile([128, Q], f16, tag="pk1", name=f"pt{h}")
                    for t in range(NT):
                        nc.tensor.transpose(
                            pt[:, 128 * t : 128 * (t + 1)],
                            acc[:, 128 * t : 128 * (t + 1)],
                            ident16,
                        )
                        if t == 7:
                            nc.vector.tensor_reduce(
                                out=cols[:, 0:8],
                                in_=pt[:, 0:1024].rearrange(
                                    "p (t c) -> p t c", c=128),
                                axis=AX.X, op=ALU.max,
                            )
                    nc.vector.tensor_reduce(
                        out=cols[:, 8:16],
                        in_=pt[:, 1024:Q].rearrange("p (t c) -> p t c", c=128),
                        axis=AX.X, op=ALU.max,
                    )
                    nc.vector.tensor_scalar_max(cols, cols, 0.0)
                    rows = rows0 if h == 0 else rows1
                    nc.vector.tensor_scalar_max(rows, rows, 0.0)

                strips(0)
                finalize(0)
                strips(1)
                finalize(1)

            # late inputs (tail only)
            m_sb = sb.tile([128, NT, C], f32, tag="m_sb")
            nc.sync.dma_start(
                out=m_sb, in_=m_d[:, :].rearrange("(t p) c -> p t c", p=128)
            )
            bm = sb.tile([128, 2], f32, tag="bm")
            nc.sync.dma_start(out=bm, in_=bm_d[:, :])

            # ---- softmax tail ----
            alpha_seg = sb.tile([128, 34], f32, tag="alpha_seg")
            s_pm = sb.tile([128, 4], f32, tag="s_pm")
            ssum = sb.tile([4, 1], f32, tag="ssum")
            srec = sb.tile([4, 1], f32, tag="srec")
            w34 = sb.tile([128, 34], f32, tag="w34")
            w2 = sb.tile([128, 17, 2], f32, tag="w2")
            r_sb = sb.tile([64, 4], f32, tag="r_sb")
            rt_sb = sb.tile([4, C], f32, tag="rt_sb")

            with tc.tile_pool(name="psF", bufs=1, space="PSUM") as psF:
                # alpha, segment-aligned cols: [h0s1 0:10 | h1s1 10:20 |
                # h0s2 20:27 | h1s2 27:34]; boundary row 1200 = tile 9 part 48
                nc.vector.tensor_add(alpha_seg[:, 0:10], rows0[:, 0:10], cols0[:, 0:10])
                nc.vector.tensor_add(alpha_seg[:, 10:20], rows1[:, 0:10], cols1[:, 0:10])
                nc.vector.tensor_add(alpha_seg[:, 20:27], rows0[:, 9:16], cols0[:, 9:16])
                nc.vector.tensor_add(alpha_seg[:, 27:34], rows1[:, 9:16], cols1[:, 9:16])
                # kill the out-of-segment halves of boundary tile 9 by adding
                # -3e38 (host mask; DVE ops cannot start at partition 48)
                nc.vector.tensor_add(alpha_seg[:, 9:10], alpha_seg[:, 9:10], bm[:, 0:1])
                nc.vector.tensor_add(alpha_seg[:, 19:20], alpha_seg[:, 19:20], bm[:, 0:1])
                nc.vector.tensor_add(alpha_seg[:, 20:21], alpha_seg[:, 20:21], bm[:, 1:2])
                nc.vector.tensor_add(alpha_seg[:, 27:28], alpha_seg[:, 27:28], bm[:, 1:2])

                # alpha >= 0 and bounded far below fp32 exp overflow for
                # randn-scale inputs, so softmax needs no max-subtraction:
                # exp(alpha)/sum is identical
                segs = [(0, 10), (10, 20), (20, 27), (27, 34)]
                for k, (a, b) in enumerate(segs):
                    nc.scalar.activation(
                        out=w34[:, a:b], in_=alpha_seg[:, a:b], func=ACTF.Exp,
                        scale=1.0,
                        accum_out=s_pm[:, k : k + 1],
                    )
                pm2 = psF.tile([128, 128], f32, tag="psmall", name="pm2")[0:4, :]
                nc.tensor.transpose(pm2[:, :], s_pm[:, :], ident32)
                nc.vector.tensor_reduce(out=ssum, in_=pm2[:, :], axis=AX.X, op=ALU.add)
                nc.vector.reciprocal(srec, ssum)

                # interleave weights so each M-tile's (h0, h1) pair is one
                # contiguous [128, 2] matmul rhs
                nc.vector.tensor_copy(w2[:, 0:10, 0], w34[:, 0:10])
                nc.vector.tensor_copy(w2[:, 0:10, 1], w34[:, 10:20])
                nc.vector.tensor_copy(w2[:, 10:17, 0], w34[:, 20:27])
                nc.vector.tensor_copy(w2[:, 10:17, 1], w34[:, 27:34])

                r1p = psF.tile([64, 2], f32, tag="r1p")
                r2p = psF.tile([64, 2], f32, tag="r2p")
                for t in range(10):
                    nc.tensor.matmul(
                        r1p[:, :], m_sb[:, t, :], w2[:, t, :],
                        start=(t == 0), stop=(t == 9),
                    )
                for t in range(7):
                    nc.tensor.matmul(
                        r2p[:, :], m_sb[:, 9 + t, :], w2[:, 10 + t, :],
                        start=(t == 0), stop=(t == 6),
                    )
                nc.vector.tensor_copy(r_sb[:, 0:2], r1p[:, :])
                nc.vector.tensor_copy(r_sb[:, 2:4], r2p[:, :])
                rtp = psF.tile([4, C], f32, tag="rtp")
                nc.tensor.transpose(rtp[:, :], r_sb[:, :], ident32[0:64, 0:64])
                nc.vector.tensor_scalar_mul(rt_sb, rtp[:, :], srec)
                nc.sync.dma_start(out=out_d[:, :], in_=rt_sb)

    return nc


def _get_nc():
    if "nc" not in _CACHE:
        _CACHE["nc"] = _patch_bass_json(_build_nc())
    return _CACHE["nc"]


def _host_inputs(x1, x2, U):
    x1 = np.asarray(x1, dtype=np.float32)
    x2 = np.asarray(x2, dtype=np.float32)
    U = np.asarray(U, dtype=np.float32)
    us = (U * (C ** -0.5)).astype(np.float32)

    p = np.arange(128)
    sa = np.zeros((128, NT), np.float32)
    ea = np.zeros((128, NT), np.float32)
    for t in range(NT):
        masked = (t * 128 + p) < MASKED
        sa[:, t] = np.where(masked, float(MASKED), 0.0)
        ea[:, t] = np.where(masked, 0.0, float(Q))
    bm = np.zeros((128, 2), np.float32)
    bm[:, 0] = np.where(p >= L1 - 9 * 128, -3.0e38, 0.0)  # seg1 tile9: kill p>=48
    bm[:, 1] = np.where(p < L1 - 9 * 128, -3.0e38, 0.0)   # seg2 tile9: kill p<48

    in_maps = []
    for b in range(B):
        x2p = np.zeros((L2, C), np.float32)
        x2p[:, :D2] = x2[b]
        M = np.concatenate([x1[b], x2p], axis=0)  # [2048, 64]
        at = np.empty((C, 2, Q), np.float32)
        at[:, 0, :] = (M @ us[0]).T
        at[:, 1, :] = (M @ us[1]).T
        in_maps.append(
            {
                "mt_in": np.ascontiguousarray(M.T),
                "m_in": np.ascontiguousarray(M),
                "at_in": at,
                "starta_in": sa,
                "enda_in": ea,
                "bmask_in": bm,
            }
        )
    return in_maps


def run_cores(x1, x2, U, **kw):
    """Run on 8 cores; returns BassKernelResults."""
    from concourse.bass_utils import run_bass_kernel_spmd

    nc = _get_nc()
    in_maps = _host_inputs(x1, x2, U)
    return run_bass_kernel_spmd(nc, in_maps, core_ids=list(range(B)), **kw)


def kernel(x1, x2, U):
    res = run_cores(x1, x2, U)
    r1 = np.zeros((B, H, C), np.float32)
    r2 = np.zeros((B, H, C), np.float32)
    for b in range(B):
        o = res.results[b]["out"]
        r1[b] = o[0:2, :]
        r2[b] = o[2:4, :]
    return r1, r2



# revision 24
# speedup vs baseline: 1.3238x; 1.3238x over previous
"""Trainium2 Bass kernel for nn_DINA_25503515804209 (sparse_attention).

Math (per batch b, head h):
  M = concat(x1, pad(x2)) in R^{2048 x 64}
  K = (1/8) * M U_h M^T          (2048 x 2048)
  rows_i = max(0, max_{p in allowed(i)} K[i,p])
  cols_p = max(0, max_{i in allowed(p)} K[i,p])
    (leading 848x848 block masked; the reference's mask fill value
     min(relu(K_head0)) is 0 for any real input since relu >= 0 and some
     entry is always <= 0 -- the max(0, .) floor implements it exactly)
  alpha = rows + cols; w1 = softmax(alpha[:1200]); w2 = softmax(alpha[1200:])
  r1 = w1 @ M[:1200]; r2 = w2 @ M[1200:]

Sharding: data-parallel over batch B=8 across the 8 NeuronCores.

Engine split per core (drain of K is the bottleneck; K never fits SBUF):
  columns [0:1024)   "Y" raw/max domain: DVE masked-reduce / Pool copy +
                     DVE pairwise rowmax, folded into a running fp16 max
                     surface (colmax) on the DVE.
  columns [1024:2048) "X" LSE domain: ACT computes exp(T*(K-B)) (bf16) with
                     the fused per-row accumulator giving the row-sums for
                     a log-sum-exp rowmax (exact to ~1e-3 for continuous
                     data); the PE folds the exp surfaces with a
                     ones-vector matmul accumulated in PSUM, giving column
                     sums; cols_X = B + ln(colsum)/T, floored at 0 (the
                     relu floor makes every under/overflow case exact).
  max(LSE, raw) combines are valid per row/column subset; T=16, B=4 keeps
  every exponent inside f32/bf16 range for randn-scale inputs.
"""

import json

import numpy as np

B, L1, D1, L2, D2, H, C = 8, 1200, 64, 848, 48, 2, 64
Q = L1 + L2            # 2048
NT = Q // 128          # 16 row tiles
MASKED = L2            # leading 848x848 block is masked
HW = 1024              # X/Y column-domain boundary
T_LSE = 16.0
B_LSE = 4.0

_CACHE = {}


# --------------------------------------------------------------------------
# BIR post-processing: this walrus build encodes at most one semaphore wait
# per instruction; Tile emits multi-wait sync_infos.  Hoist excess waits
# into preceding same-engine EventSemaphore instructions (what wait_ge
# emits) -- engine sequencers execute in order, so semantics are identical.
# Also run codegen_inst_isa_subclasses, which populates .instr bytes for
# InstISA subclasses (custom DVE ops); raw Bass does not run that pass and
# walrus fails with "ISA wrong length" on empty instr arrays.
# --------------------------------------------------------------------------
def _split_waits_json(j):
    for fn in j.get("functions", []):
        for blk in fn.get("blocks", []):
            insts = blk.get("instructions")
            if not insts:
                continue
            out = []
            for ins in insts:
                si = ins.get("sync_info")
                waits = (si or {}).get("on_wait") or []
                if len(waits) > 1:
                    for k, wt in enumerate(waits[:-1]):
                        out.append(
                            {
                                "debug": ins.get("debug"),
                                "engine": ins["engine"],
                                "ins": [],
                                "name": f"{ins['name']}_hw{k}",
                                "opcode": "EventSemaphore",
                                "outs": [],
                                "sync_info": {"on_update": [], "on_wait": [wt]},
                            }
                        )
                    si["on_wait"] = waits[-1:]
                ups = (si or {}).get("on_update") or []
                if len(ups) > 1:
                    raise RuntimeError(
                        f"instruction {ins['name']} has {len(ups)} updates"
                    )
                out.append(ins)
            blk["instructions"] = out


def _patch_bass_json(nc):
    import concourse.mybir as mybir

    orig = nc.to_json_bytes
    done = []

    def to_json_bytes_patched():
        if not done:
            mybir.codegen_inst_isa_subclasses(nc)
            done.append(True)
        j = json.loads(orig())
        _split_waits_json(j)
        return json.dumps(j).encode()

    nc.to_json_bytes = to_json_bytes_patched
    return nc


def _ttmax_reduce_op():
    """Fused  out = max(in0, in1);  accum_out = rowmax(out)  custom DVE op.

    Consumes two fp16 streams per cycle (both DVE read ports), so one
    instruction replaces the whole pairwise row-max tree of a strip."""
    import numpy as np
    import concourse.dve_ops as dve_ops
    from concourse.dve_spec import Spec, Src0, Src1, maxx, lower
    from concourse.dve_table_gen import dve_ver_for
    from concourse.dve_uop import DveOpSpec

    NAME = "TT_MAX_ROWMAX_ANT"
    if NAME in dve_ops._SUB_OPCODE_FOR_NAME:
        return next(op for op in dve_ops.OPS if op.name == NAME)

    def _ref(in0, in1, c0, c1, c2):
        body = np.maximum(in0.astype(np.float32), in1.astype(np.float32))
        return body, body.reshape(body.shape[0], -1).max(axis=-1, keepdims=True)

    spec = Spec(body=maxx(Src0, Src1), accum=maxx, reference=_ref)
    row = dve_ops._CUSTOM_DVE_ROW_BASE + len(dve_ops.OPS)
    ver = dve_ver_for("TRN2")
    sha = DveOpSpec(
        name=NAME, opcode=row, uops=lower(spec, ver=ver), rd1_en=True
    ).sha(ver)
    op = dve_ops.DveOp(NAME, spec, subdim=False, uops_sha={ver: sha})
    dve_ops.OPS.append(op)
    dve_ops._SUB_OPCODE_FOR_NAME[NAME] = row
    dve_ops.CUSTOM_DVE_SPECS[NAME] = spec
    return op


def _build_nc():
    import concourse.bass as bass
    import concourse.mybir as mybir
    import concourse.tile as tile
    from concourse.dve_ops import TENSOR_MASK_REDUCE
    from concourse.masks import make_identity

    ttmax = _ttmax_reduce_op()

    f32 = mybir.dt.float32
    f32r = mybir.dt.float32r
    f16 = mybir.dt.float16
    bf16 = mybir.dt.bfloat16
    AX = mybir.AxisListType
    ALU = mybir.AluOpType
    ACTF = mybir.ActivationFunctionType

    nc = bass.Bass(trn_type="TRN2")

    mt_d = nc.dram_tensor("mt_in", [C, Q], bf16, kind="ExternalInput")
    m_d = nc.dram_tensor("m_in", [Q, C], f32, kind="ExternalInput")
    at_d = nc.dram_tensor("at_in", [C, 2, Q], bf16, kind="ExternalInput")
    sk_d = nc.dram_tensor("skew_in", [128, 3], f32, kind="ExternalInput")
    out_d = nc.dram_tensor("out", [4, C], f32, kind="ExternalOutput")

    NRESTR = 6          # strips 0..5 fully inside the masked row block
    # interleave restricted (ACT-heavy) and normal (DVE-heavy) strips
    ORDER = [0, 7, 1, 8, 2, 9, 3, 10, 4, 11, 5, 12, 6, 13, 14, 15]
    ACT_COPY = {13, 15}  # normal strips whose raw copy runs on ACT not DVE

    with tile.TileContext(nc) as tc:
        with (
            tc.tile_pool(name="sb", bufs=1) as sb,
            tc.tile_pool(name="ab", bufs=4) as abp,
            tc.tile_pool(name="xb", bufs=4) as xbp,
        ):
            # ---- load inputs (f32r tiles loaded directly; PE rounds).
            # A^T = (M U_h)^T is precomputed on the host so strip matmuls
            # start as soon as the first DMA chunks land.  Strip 0 is
            # restricted: cols [848:2048) first.
            mtr = sb.tile([C, Q], bf16, tag="mtr")
            atr = sb.tile([C, 2, Q], bf16, tag="atr")
            nc.scalar.dma_start(
                out=atr[:, :, 0:128], in_=at_d[:, :, 0:128]
            )
            for j in (1, 2, 3, 0):
                s = slice(512 * j, 512 * (j + 1))
                nc.sync.dma_start(out=mtr[:, s], in_=mt_d[:, s])
            nc.scalar.dma_start(
                out=atr[:, :, 128:1024], in_=at_d[:, :, 128:1024]
            )
            for j in (2, 3):
                s = slice(512 * j, 512 * (j + 1))
                nc.scalar.dma_start(out=atr[:, :, s], in_=at_d[:, :, s])

            skew = sb.tile([128, 3], f32, tag="skew")
            nc.sync.dma_start(out=skew, in_=sk_d[:, :])
            sa6 = skew[:, 0:1]           # 848 for p<80 else 0 (boundary strip)

            e1024 = sb.tile([128, 1], f32, tag="e1024")
            nc.vector.memset(e1024, float(HW))
            e176 = sb.tile([128, 1], f32, tag="e176")
            nc.vector.memset(e176, 176.0)
            ebias = sb.tile([128, 1], f32, tag="ebias")
            nc.vector.memset(ebias, -T_LSE * B_LSE)
            ones_bf = sb.tile([128, 8], bf16, tag="ones_bf")
            nc.vector.memset(ones_bf, 1.0)

            ident16 = sb.tile([128, 128], f16, tag="ident16")
            make_identity(nc, ident16)
            ident32 = sb.tile([128, 128], f32, tag="ident32")
            make_identity(nc, ident32)

            # per-head result surfaces
            yrows = [sb.tile([128, NT], f32, tag=f"yrows{h}", name=f"yrows{h}") for h in (0, 1)]
            xrs = [sb.tile([128, NT], f32, tag=f"xrs{h}", name=f"xrs{h}") for h in (0, 1)]
            rows = [sb.tile([128, NT], f32, tag=f"rows{h}", name=f"rows{h}") for h in (0, 1)]
            cols = [sb.tile([128, NT], f32, tag=f"cols{h}", name=f"cols{h}") for h in (0, 1)]
            yacc = [sb.tile([128, HW], f16, tag=f"yacc{h}", name=f"yacc{h}") for h in (0, 1)]
            scr = sb.tile([128, 512], f16, tag="scr")   # ttmax body dump

            with (
                tc.tile_pool(name="psR", bufs=1, space="PSUM") as psR,
                tc.tile_pool(name="psC", bufs=1, space="PSUM") as psC,
            ):
                def strips(h):
                    nc.vector.memset(yacc[h][:, :], -3.0e38)
                    nc.vector.memset(yrows[h][:, :], -3.0e38)
                    csb = psC.tile([128, HW], f32, tag="csb", name=f"csb{h}")
                    first = [True]
                    _CSB[h] = csb
                    for k, t in enumerate(ORDER):
                        isl = slice(128 * t, 128 * (t + 1))
                        restr = t < NRESTR
                        rA = psR.tile([128, HW], f32, tag=f"r{(2 * k) % 3}",
                                      name=f"rA_{h}_{t}")
                        rB = psR.tile([128, HW], f32, tag=f"r{(2 * k + 1) % 3}",
                                      name=f"rB_{h}_{t}")
                        # K strip: A-part cols [0:1024), B-part [1024:2048)
                        for j in range(2):
                            if restr and j == 0:
                                continue
                            nc.tensor.matmul(
                                rA[:, 512 * j : 512 * (j + 1)],
                                atr[:, h, isl],
                                mtr[:, 512 * j : 512 * (j + 1)],
                                start=True, stop=True,
                            )
                        for j in range(2):
                            nc.tensor.matmul(
                                rB[:, 512 * j : 512 * (j + 1)],
                                atr[:, h, isl],
                                mtr[:, 1024 + 512 * j : 1024 + 512 * (j + 1)],
                                start=True, stop=True,
                            )
                        # ---- X domain: exp + fused row-sum on ACT, then a
                        # ones-matmul accumulates column sums into csb.
                        xbuf = xbp.tile([128, HW], bf16, tag="x",
                                        name=f"x_{h}_{t}")
                        nc.scalar.activation(
                            out=xbuf, in_=rB[:, :], func=ACTF.Exp,
                            scale=T_LSE, bias=ebias,
                            accum_out=xrs[h][:, t : t + 1],
                        )
                        for j in range(2):
                            nc.tensor.matmul(
                                csb[0:8, 512 * j : 512 * (j + 1)],
                                ones_bf, xbuf[:, 512 * j : 512 * (j + 1)],
                                start=first[0], stop=(k == NT - 1),
                            )
                        first[0] = False
                        # ---- Y domain (raw max): copy + rowmax + fold
                        if restr:
                            # live Y part is only cols [848:1024)
                            rbuf = abp.tile([128, 176], f16, tag="ar",
                                            name=f"ar_{h}_{t}")
                            nc.vector._custom_dve(
                                TENSOR_MASK_REDUCE,
                                out=rbuf, in0=rA[:, 848:HW], in1=e176,
                                s0=0.0, s1=0.0, imm2=1.0,
                                accum_out=yrows[h][:, t : t + 1],
                            )
                            nc.vector.tensor_max(
                                yacc[h][:, 848:HW], yacc[h][:, 848:HW], rbuf
                            )
                        elif t == NRESTR:
                            # boundary strip: per-partition column mask
                            bbuf = abp.tile([128, HW], f16, tag="a",
                                            name=f"a_{h}_{t}")
                            nc.vector._custom_dve(
                                TENSOR_MASK_REDUCE,
                                out=bbuf, in0=rA[:, :], in1=e1024,
                                s0=sa6, s1=0.0, imm2=1.0,
                                accum_out=yrows[h][:, t : t + 1],
                            )
                            nc.vector.tensor_max(yacc[h], yacc[h], bbuf)
                        elif t in ACT_COPY:
                            # ACT copies PSUM->fp16; DVE pairwise rowmax+fold
                            abuf = abp.tile([128, HW], f16, tag="a",
                                            name=f"a_{h}_{t}")
                            nc.scalar.copy(abuf, rA[:, :])
                            nc.vector._custom_dve(
                                ttmax,
                                out=scr, in0=abuf[:, 0:512],
                                in1=abuf[:, 512:HW],
                                accum_out=yrows[h][:, t : t + 1],
                            )
                            nc.vector.tensor_max(yacc[h], yacc[h], abuf)
                        else:
                            # normal strip: one DVE masked-reduce gives the
                            # fp16 copy and the rowmax in a single pass
                            abuf = abp.tile([128, HW], f16, tag="a",
                                            name=f"a_{h}_{t}")
                            nc.vector._custom_dve(
                                TENSOR_MASK_REDUCE,
                                out=abuf, in0=rA[:, :], in1=e1024,
                                s0=0.0, s1=0.0, imm2=1.0,
                                accum_out=yrows[h][:, t : t + 1],
                            )
                            nc.vector.tensor_max(yacc[h], yacc[h], abuf)

                def dump_csb(h):
                    # the ones-matmul wrote 8 identical colsum rows; copy to
                    # SBUF so the per-tile transpose matmuls can read them
                    csb8 = sb.tile([8, HW], f32, tag=f"csb8_{h}",
                                   name=f"csb8{h}")
                    nc.scalar.copy(csb8, _CSB[h][0:8, :])
                    return csb8

                def rows_fin(h):
                    # rows: max(raw Y rowmax, B + ln(rowsum_X)/T, 0); runs on
                    # ACT/DVE only, so it overlaps the other head's strips
                    lnR = sb.tile([128, NT], f32, tag="lnR", name=f"lnR{h}")
                    nc.scalar.activation(
                        out=lnR, in_=xrs[h][:, :], func=ACTF.Ln, scale=1.0
                    )
                    nc.vector.tensor_scalar(
                        out=rows[h][:, :], in0=lnR,
                        scalar1=1.0 / T_LSE, scalar2=B_LSE,
                        op0=ALU.mult, op1=ALU.add,
                    )
                    nc.vector.tensor_max(rows[h], rows[h], yrows[h])
                    nc.vector.tensor_scalar_max(rows[h], rows[h], 0.0)

                def finalize(h, csb8, ytag, xtag):
                    # --- Y columns: transpose the fp16 max surface and
                    # max-reduce over partitions.
                    ptY = psR.tile([128, HW], f16, tag=ytag, name=f"ptY{h}")
                    for k in range(HW // 128):
                        nc.tensor.transpose(
                            ptY[:, 128 * k : 128 * (k + 1)],
                            yacc[h][:, 128 * k : 128 * (k + 1)],
                            ident16,
                        )
                    # --- X columns: extract [128, 8] colsums with 8 tiny
                    # transpose-matmuls (lhsT = one 128-block of the colsum
                    # row; all 8 replicated rows are identical)
                    csbt = psR.tile([128, 8], f32, tag=xtag, name=f"csbt{h}")
                    for g in range(8):
                        nc.tensor.matmul(
                            csbt[:, g : g + 1],
                            csb8[:, 128 * g : 128 * (g + 1)],
                            ident32[0:8, 0:1],
                            start=True, stop=True,
                        )
                    csY = sb.tile([128, 8], f32, tag=f"csY{h}", name=f"csY{h}")
                    nc.vector.tensor_reduce(
                        out=csY,
                        in_=ptY[:, :].rearrange("p (t c) -> p t c", c=128),
                        axis=AX.X, op=ALU.max,
                    )
                    nc.vector.tensor_scalar_max(cols[h][:, 0:8], csY, 0.0)
                    lnB = sb.tile([128, 8], f32, tag="lnB", name=f"lnB{h}")
                    nc.scalar.activation(
                        out=lnB, in_=csbt[:, :], func=ACTF.Ln, scale=1.0
                    )
                    nc.vector.tensor_scalar(
                        out=cols[h][:, 8:16], in0=lnB,
                        scalar1=1.0 / T_LSE, scalar2=B_LSE,
                        op0=ALU.mult, op1=ALU.add,
                    )
                    nc.vector.tensor_scalar_max(
                        cols[h][:, 8:16], cols[h][:, 8:16], 0.0
                    )

                _CSB = {}
                strips(0)
                csb8_0 = dump_csb(0)
                rows_fin(0)
                strips(1)
                csb8_1 = dump_csb(1)
                rows_fin(1)
                # all finalize PE work sits after both heads' strip matmuls
                # so the PE FIFO never blocks the strip pipeline on drains
                finalize(0, csb8_0, "r0", "r2")
                finalize(1, csb8_1, "r1", "r2")

                # late inputs (tail only)
                m_sb = sb.tile([128, NT, C], f32, tag="m_sb")
                nc.sync.dma_start(
                    out=m_sb, in_=m_d[:, :].rearrange("(t p) c -> p t c", p=128)
                )

                # ---- softmax tail ----
                bm = skew[:, 1:3]
                alpha_seg = sb.tile([128, 34], f32, tag="alpha_seg")
                s_pm = sb.tile([128, 4], f32, tag="s_pm")
                ssum = sb.tile([4, 1], f32, tag="ssum")
                srec = sb.tile([4, 1], f32, tag="srec")
                w34 = sb.tile([128, 34], f32, tag="w34")
                w2 = sb.tile([128, 17, 2], f32, tag="w2")
                r_sb = sb.tile([64, 4], f32, tag="r_sb")
                rt_sb = sb.tile([4, C], f32, tag="rt_sb")

                # alpha, segment-aligned cols: [h0s1 0:10 | h1s1 10:20 |
                # h0s2 20:27 | h1s2 27:34]; boundary row 1200 = tile 9 part 48
                nc.vector.tensor_add(alpha_seg[:, 0:10], rows[0][:, 0:10], cols[0][:, 0:10])
                nc.vector.tensor_add(alpha_seg[:, 10:20], rows[1][:, 0:10], cols[1][:, 0:10])
                nc.vector.tensor_add(alpha_seg[:, 20:27], rows[0][:, 9:16], cols[0][:, 9:16])
                nc.vector.tensor_add(alpha_seg[:, 27:34], rows[1][:, 9:16], cols[1][:, 9:16])
                # kill the out-of-segment halves of boundary tile 9 by adding
                # -3e38 (host mask; DVE ops cannot start at partition 48)
                nc.vector.tensor_add(alpha_seg[:, 9:10], alpha_seg[:, 9:10], bm[:, 0:1])
                nc.vector.tensor_add(alpha_seg[:, 19:20], alpha_seg[:, 19:20], bm[:, 0:1])
                nc.vector.tensor_add(alpha_seg[:, 20:21], alpha_seg[:, 20:21], bm[:, 1:2])
                nc.vector.tensor_add(alpha_seg[:, 27:28], alpha_seg[:, 27:28], bm[:, 1:2])

                # alpha >= 0 and bounded far below fp32 exp overflow for
                # randn-scale inputs, so softmax needs no max-subtraction:
                # exp(alpha)/sum is identical
                segs = [(0, 10), (10, 20), (20, 27), (27, 34)]
                for k, (a, b) in enumerate(segs):
                    nc.scalar.activation(
                        out=w34[:, a:b], in_=alpha_seg[:, a:b], func=ACTF.Exp,
                        scale=1.0,
                        accum_out=s_pm[:, k : k + 1],
                    )
                pm2 = psR.tile([128, 128], f32, tag="r0", name="pm2")[0:4, :]
                nc.tensor.transpose(pm2[:, :], s_pm[:, :], ident32)
                nc.vector.tensor_reduce(out=ssum, in_=pm2[:, :], axis=AX.X, op=ALU.add)
                nc.vector.reciprocal(srec, ssum)

                # interleave weights so each M-tile's (h0, h1) pair is one
                # contiguous [128, 2] matmul rhs
                nc.vector.tensor_copy(w2[:, 0:10, 0], w34[:, 0:10])
                nc.vector.tensor_copy(w2[:, 0:10, 1], w34[:, 10:20])
                nc.vector.tensor_copy(w2[:, 10:17, 0], w34[:, 20:27])
                nc.vector.tensor_copy(w2[:, 10:17, 1], w34[:, 27:34])

                r1p = psR.tile([64, 2], f32, tag="r1", name="r1p")
                r2p = psR.tile([64, 2], f32, tag="r2", name="r2p")
                for t in range(10):
                    nc.tensor.matmul(
                        r1p[:, :], m_sb[:, t, :], w2[:, t, :],
                        start=(t == 0), stop=(t == 9),
                    )
                for t in range(7):
                    nc.tensor.matmul(
                        r2p[:, :], m_sb[:, 9 + t, :], w2[:, 10 + t, :],
                        start=(t == 0), stop=(t == 6),
                    )
                nc.vector.tensor_copy(r_sb[:, 0:2], r1p[:, :])
                nc.vector.tensor_copy(r_sb[:, 2:4], r2p[:, :])
                rtp = psR.tile([4, C], f32, tag="r0", name="rtp")
                nc.tensor.transpose(rtp[:, :], r_sb[:, :], ident32[0:64, 0:64])
                nc.vector.tensor_scalar_mul(rt_sb, rtp[:, :], srec)
                nc.sync.dma_start(out=out_d[:, :], in_=rt_sb)

    return nc


def _get_nc():
    if "nc" not in _CACHE:
        _CACHE["nc"] = _patch_bass_json(_build_nc())
    return _CACHE["nc"]


def _host_inputs(x1, x2, U):
    x1 = np.asarray(x1, dtype=np.float32)
    x2 = np.asarray(x2, dtype=np.float32)
    U = np.asarray(U, dtype=np.float32)
    us = (U * (C ** -0.5)).astype(np.float32)

    p = np.arange(128)
    skew = np.zeros((128, 3), np.float32)
    skew[:, 0] = np.where(p < 80, float(MASKED), 0.0)      # boundary strip mask
    skew[:, 1] = np.where(p >= L1 - 9 * 128, -3.0e38, 0.0)  # seg1 tile9: kill p>=48
    skew[:, 2] = np.where(p < L1 - 9 * 128, -3.0e38, 0.0)   # seg2 tile9: kill p<48

    from ml_dtypes import bfloat16

    in_maps = []
    for b in range(B):
        x2p = np.zeros((L2, C), np.float32)
        x2p[:, :D2] = x2[b]
        M = np.concatenate([x1[b], x2p], axis=0)  # [2048, 64]
        at = np.empty((C, 2, Q), np.float32)
        at[:, 0, :] = (M @ us[0]).T
        at[:, 1, :] = (M @ us[1]).T
        in_maps.append(
            {
                "mt_in": np.ascontiguousarray(M.T).astype(bfloat16),
                "m_in": np.ascontiguousarray(M),
                "at_in": at.astype(bfloat16),
                "skew_in": skew,
            }
        )
    return in_maps


def run_cores(x1, x2, U, **kw):
    """Run on 8 cores; returns BassKernelResults."""
    from concourse.bass_utils import run_bass_kernel_spmd

    nc = _get_nc()
    in_maps = _host_inputs(x1, x2, U)
    return run_bass_kernel_spmd(nc, in_maps, core_ids=list(range(B)), **kw)


def kernel(x1, x2, U):
    res = run_cores(x1, x2, U)
    r1 = np.zeros((B, H, C), np.float32)
    r2 = np.zeros((B, H, C), np.float32)
    for b in range(B):
        o = res.results[b]["out"]
        r1[b] = o[0:2, :]
        r2[b] = o[2:4, :]
    return r1, r2
